# revision 35
# baseline (speedup 1.0000x reference)
"""BiasPredictLoss Trainium2 kernel.

Data-parallel over batch: 8 samples -> 8 NeuronCores, one sample each.
Per core computes the per-sample sum of squared errors (SSE) of
(b - b_new); host averages the 8 scalars.

Math (per sample, K = 17x17 separable Gaussian, sigma=4, p=2):
  mask  = (I > 0)
  r     = 1/(conv(mask)+EPS)            (ACT: exp(-ln(Kb+EPS)), psum-direct)
  t1    = r*I   (mask*I == I);  t2 = r*mask
  A1    = conv(b)*t1 ; A2 = conv(b^2)*t2
  num_c = sum(u_c^2*A1); den_c = sum(u_c^2*A2); v_c = num/(den+EPS)
  w1    = sum_c u_c^2 v_c ; w2 = sum_c u_c^2 v_c^2
  q     = conv(I*w1)/conv(w2)           (valid where mask==1; conv(w2)>0
                                         everywhere since u>0 regardless of mask)
  e     = b - q   where mask==1, else b - 1    (reference's EPS terms are
                                                f32-invisible; masked branch exact)
  SSE   = sum(e^2)

Convolution on TensorE in bf16 (fp32 matmuls decompose 2.3x on trn2):
  conv2(X^T) = Ag @ ((X^T)^T @ Ag) with the image as the stationary operand in
  pass 1 -- conv of a transposed input returns a normal-orientation output, so
  phase-A and phase-B convs all take transposed bf16 inputs and everything else
  stays in normal orientation.
"""

import sys

import numpy as np

for _p in ("/opt/trn_rl_repo",):
    if _p not in sys.path:
        sys.path.insert(0, _p)

import concourse.bass as bass
import concourse.mybir as mybir
from concourse.tile import TileContext
from concourse.bass_utils import run_bass_kernel_spmd

F32 = mybir.dt.float32
BF16 = mybir.dt.bfloat16
OP = mybir.AluOpType
AF = mybir.ActivationFunctionType

EPS = 1e-9
H = W = 512
NCH = 4
NB = 4  # 128-row blocks per image
NCORES = 8
SIG = 4
KS = 4 * SIG + 1
HB = KS // 2


def _toeplitz(dtype):
    ax = np.arange(KS, dtype=np.float64) - (KS - 1) / 2.0
    g = np.exp(-(ax ** 2) / (2.0 * SIG ** 2))
    gn = g / g.sum()
    A = np.zeros((H, H), dtype=np.float64)
    for t in range(-HB, HB + 1):
        v = gn[t + HB]
        idx = np.arange(max(0, -t), min(H, H - t))
        A[idx, idx + t] = v
    return A.astype(dtype)


def _blk(t, j):
    return t[:, j * 512:(j + 1) * 512]


def _sub(t, j, m):
    return t[:, j * 512 + m * 128: j * 512 + m * 128 + 128]


DEBUG_TAPS = False


def build_nc():
    import ml_dtypes
    nc = bass.Bass()
    I_ext = nc.declare_dram_parameter("I", [H, W], F32, isOutput=False)
    u_ext = nc.declare_dram_parameter("u", [NCH, H, W], F32, isOutput=False)
    b_ext = nc.declare_dram_parameter("b", [H, W], F32, isOutput=False)
    out_ext = nc.declare_dram_parameter("out", [1, 1], F32, isOutput=True)

    dbg_exts = {}
    if DEBUG_TAPS:
        for nm, shp, dt in [("d_acc", [128, 9], F32), ("d_nd", [1, 16], F32),
                            ("d_vb", [128, 8], F32), ("d_e", [128, 2048], F32),
                            ("d_r", [128, 2048], F32),
                            ("d_q", [128, 2048], F32)]:
            dbg_exts[nm] = nc.declare_dram_parameter(nm, shp, dt, isOutput=True)

    # register extra const APs used by ACT bias lowering (same pattern as
    # Bass.__init__'s builtins)
    _cm1 = nc.alloc_sbuf_tensor("const-float32-m1", [128, 1], F32)
    nc.gpsimd.memset(_cm1.ap(), -1.0)
    nc.const_aps.aps[(F32, -1.0)] = _cm1.ap()
    nc.all_engine_barrier()

    Ag_d = nc.inline_tensor(_toeplitz(ml_dtypes.bfloat16), name="Ag_const")
    id_d = nc.inline_tensor(np.eye(128, dtype=ml_dtypes.bfloat16),
                            name="id_const")
    onec_d = nc.inline_tensor(np.ones((128, 1), np.float32), name="onec_const")
    onecb_d = nc.inline_tensor(np.ones((128, 1), ml_dtypes.bfloat16),
                               name="onecb_const")
    oner_d = nc.inline_tensor(np.ones((1, 128), np.float32), name="oner_const")

    with TileContext(nc) as tc:
        with tc.tile_pool(name="const", bufs=1) as cpool, \
             tc.tile_pool(name="imgs", bufs=1) as ipool, \
             tc.tile_pool(name="ps", bufs=1, space="PSUM") as pspool:

            # ---- constants to SBUF ----
            Ag = cpool.tile([128, 2048], BF16, tag="Ag")
            nc.sync.dma_start(
                out=Ag[:].rearrange("p (j w) -> p j w", w=512),
                in_=Ag_d[:].rearrange("(j p) w -> p j w", p=128))
            ident = cpool.tile([128, 128], BF16, tag="ident")
            nc.sync.dma_start(out=ident[:], in_=id_d[:])
            onec = cpool.tile([128, 1], F32, tag="onec")
            nc.sync.dma_start(out=onec[:], in_=onec_d[:])
            onecb_raw = cpool.tile([128, 1], BF16, tag="onecb_raw")
            nc.sync.dma_start(out=onecb_raw[:], in_=onecb_d[:])
            onecb = cpool.tile([128, 1], BF16, tag="onecb")
            nc.vector.tensor_copy(onecb[:], onecb_raw[:])
            oner = cpool.tile([1, 128], F32, tag="oner")
            nc.sync.dma_start(out=oner[:], in_=oner_d[:])
            eps_col = cpool.tile([128, 1], F32, tag="eps_col")
            nc.vector.memset(eps_col[:], EPS)

            def tap(nm, tile_ap):
                if DEBUG_TAPS:
                    nc.sync.dma_start(out=dbg_exts[nm][:], in_=tile_ap)

            def _half_conv(X_bf, outtag):
                """one data-stationary pass: out = X^T @ Ag (windowed band)."""
                out = pspool.tile([128, 2048], F32, tag=outtag, name=outtag)
                for m in range(NB):
                    for k in range(NB):
                        n0 = max(0, k * 128 - HB)
                        n1 = min(512, k * 128 + 128 + HB)
                        nc.tensor.matmul(
                            out[:, m * 512 + n0: m * 512 + n1],
                            lhsT=_sub(X_bf, k, m),
                            rhs=Ag[:, k * 512 + n0: k * 512 + n1],
                            start=(k == 0), stop=(k == NB - 1))
                return out

            def conv2(X_bf, outtag):
                """X normal bf16 -> conv2(X) psum f32, normal orientation.
                P1 = X^T@Ag = (AgX)^T, then out = P1^T@Ag = (AgX)Ag."""
                p1 = _half_conv(X_bf, "p1ps")
                p1sb = ipool.tile([128, 2048], BF16, tag="p1sb")
                nc.any.tensor_copy(p1sb[:], p1[:])
                return _half_conv(p1sb, outtag)

            # ---- input DMA ----
            I_sb = ipool.tile([128, 2048], F32, tag="I")
            b_sb = ipool.tile([128, 2048], F32, tag="b")
            u_sb = [ipool.tile([128, 2048], F32, tag=f"u{c}", name=f"u{c}")
                    for c in range(NCH)]
            # I first (gates mask -> conv(mask) -> r, the critical chain),
            # then b, then u (only needed from the squares onward)
            for j in range(NB):
                nc.sync.dma_start(out=_blk(I_sb, j), in_=I_ext[j * 128:(j + 1) * 128, :])
            for j in range(NB):
                nc.sync.dma_start(out=_blk(b_sb, j), in_=b_ext[j * 128:(j + 1) * 128, :])
            for c in range(NCH):
                for j in range(NB):
                    nc.sync.dma_start(out=_blk(u_sb[c], j),
                                      in_=u_ext[c, j * 128:(j + 1) * 128, :])

            # ---- bf16 prep + transposes ----
            mask_bf = ipool.tile([128, 2048], BF16, tag="mask_bf")
            nc.scalar.activation(mask_bf[:], I_sb[:], AF.Sign)
            b_bf = ipool.tile([128, 2048], BF16, tag="b_bf")
            nc.vector.tensor_copy(b_bf[:], b_sb[:])

            b2_bf = ipool.tile([128, 2048], BF16, tag="b2_bf")
            nc.scalar.activation(b2_bf[:], b_sb[:], AF.Square)

            # ---- phase A convolutions + r ----
            KbP = conv2(mask_bf, "convout")
            rln = ipool.tile([128, 2048], F32, tag="rln")
            nc.scalar.activation(rln[:], KbP[:], AF.Ln, bias=eps_col[:])
            r_bf = ipool.tile([128, 2048], BF16, tag="r_bf")
            nc.scalar.activation(r_bf[:], rln[:], AF.Exp, scale=-1.0)
            tap('d_r', rln[:])
            t1 = ipool.tile([128, 2048], BF16, tag="t1")
            nc.vector.tensor_mul(t1[:], r_bf[:], I_sb[:])
            t2 = ipool.tile([128, 2048], BF16, tag="t2")
            nc.vector.tensor_mul(t2[:], r_bf[:], mask_bf[:])

            CbP = conv2(b_bf, "convout")
            A1 = ipool.tile([128, 2048], BF16, tag="A1")
            nc.vector.tensor_mul(A1[:], CbP[:], t1[:])
            Cb2P = conv2(b2_bf, "convout")
            A2 = ipool.tile([128, 2048], BF16, tag="A2")
            nc.vector.tensor_mul(A2[:], Cb2P[:], t2[:])

            # ---- u squares (bf16) ----
            s_sb = []
            for c in range(NCH):
                s = ipool.tile([128, 2048], BF16, tag=f"s{c}", name=f"s{c}")
                nc.scalar.activation(s[:], u_sb[c][:], AF.Square)
                s_sb.append(s)

            # ---- class-center reductions ----
            # nf_c = s_c * A (bf16 TT, 2x) then TensorE ones-matmul reduces
            # partitions into [1,512] psum rows (keeps PE warm mid-kernel);
            # one batched 3D tensor_reduce per psum tile finishes the job.
            acc9 = cpool.tile([128, 9], F32, tag="acc9")
            nd = cpool.tile([1, 16], F32, tag="nd")
            redN = pspool.tile([128, 2048], F32, tag="p1ps")
            redD = pspool.tile([128, 2048], F32, tag="convout")
            for c in range(NCH):
                nf = ipool.tile([128, 2048], BF16, tag=f"nf{c & 1}",
                                name=f"nf{c}")
                nc.vector.tensor_mul(nf[:], s_sb[c][:], A1[:])
                for j in range(NB):
                    nc.tensor.matmul(redN[0:1, c * 512:(c + 1) * 512],
                                     lhsT=onecb[:], rhs=_blk(nf, j),
                                     start=(j == 0), stop=(j == NB - 1))
            for c in range(NCH):
                nf = ipool.tile([128, 2048], BF16, tag=f"nf{c & 1}",
                                name=f"nfd{c}")
                nc.vector.tensor_mul(nf[:], s_sb[c][:], A2[:])
                for j in range(NB):
                    nc.tensor.matmul(redD[0:1, c * 512:(c + 1) * 512],
                                     lhsT=onecb[:], rhs=_blk(nf, j),
                                     start=(j == 0), stop=(j == NB - 1))
            nc.vector.tensor_reduce(
                nd[0:1, 0:4], redN[0:1, :].rearrange("p (c n) -> p c n", n=512),
                mybir.AxisListType.X, OP.add)
            nc.vector.tensor_reduce(
                nd[0:1, 4:8], redD[0:1, :].rearrange("p (c n) -> p c n", n=512),
                mybir.AxisListType.X, OP.add)
            nc.vector.tensor_scalar_add(nd[0:1, 4:8], nd[0:1, 4:8], EPS)
            nc.vector.reciprocal(nd[0:1, 8:12], nd[0:1, 4:8])
            nc.vector.tensor_mul(nd[0:1, 12:16], nd[0:1, 0:4], nd[0:1, 8:12])
            tap('d_nd', nd[:])
            tap('d_acc', acc9[:])
            vcat = cpool.tile([1, 8], F32, tag="vcat")
            nc.vector.tensor_copy(vcat[0:1, 0:4], nd[0:1, 12:16])
            nc.vector.tensor_mul(vcat[0:1, 4:8], nd[0:1, 12:16], nd[0:1, 12:16])

            vbP = pspool.tile([128, 2048], F32, tag="convout")
            nc.tensor.matmul(vbP[:, 0:8], lhsT=oner[:], rhs=vcat[:],
                             start=True, stop=True)
            vb = cpool.tile([128, 8], F32, tag="vb")
            nc.vector.tensor_copy(vb[:], vbP[:, 0:8])
            tap('d_vb', vb[:])
            vId = cpool.tile([128, 1024], BF16, tag="vId")
            for c in range(8):
                nc.vector.tensor_scalar_mul(vId[:, c * 128:(c + 1) * 128],
                                            ident[:], vb[:, c:c + 1])

            # ---- w1 / w2 -> phase-B conv inputs (transposed bf16) ----
            w1P = pspool.tile([128, 2048], F32, tag="p1ps")
            for j in range(NB):
                for c in range(NCH):
                    nc.tensor.matmul(_blk(w1P, j), lhsT=vId[:, c * 128:(c + 1) * 128],
                                     rhs=_blk(s_sb[c], j),
                                     start=(c == 0), stop=(c == 3))
            X1 = ipool.tile([128, 2048], BF16, tag="X1")
            nc.vector.tensor_mul(X1[:], w1P[:], I_sb[:])
            w2P = pspool.tile([128, 2048], F32, tag="convout")
            for j in range(NB):
                for c in range(NCH):
                    nc.tensor.matmul(_blk(w2P, j),
                                     lhsT=vId[:, 512 + c * 128: 512 + (c + 1) * 128],
                                     rhs=_blk(s_sb[c], j),
                                     start=(c == 0), stop=(c == 3))
            X2 = ipool.tile([128, 2048], BF16, tag="X2")
            nc.any.tensor_copy(X2[:], w2P[:])

            # ---- phase B ----
            C2P = conv2(X2, "convout")
            dln = ipool.tile([128, 2048], F32, tag="dln")
            nc.scalar.activation(dln[:], C2P[:], AF.Ln)
            rDB = ipool.tile([128, 2048], F32, tag="rDB")
            nc.scalar.activation(rDB[:], dln[:], AF.Exp, scale=-1.0)
            C1P = conv2(X1, "convout")
            q = ipool.tile([128, 2048], F32, tag="q")
            nc.vector.tensor_mul(q[:], C1P[:], rDB[:])
            tap('d_q', q[:])

            e = ipool.tile([128, 2048], F32, tag="e")
            nc.vector.tensor_sub(e[:], b_sb[:], q[:])
            # masked-out pixels: e = b - 1 exactly
            z_bf = ipool.tile([128, 2048], mybir.dt.uint8, tag="z_bf")
            nc.vector.tensor_scalar(z_bf[:], mask_bf[:], 0.0, None,
                                    OP.is_equal)
            bm1 = ipool.tile([128, 2048], F32, tag="bm1")
            nc.scalar.add(bm1[:], b_sb[:], -1.0)
            nc.vector.copy_predicated(e[:], z_bf[:], bm1[:])
            tap('d_e', e[:])

            junk2 = ipool.tile([128, 2048], F32, tag="junk2")
            nc.vector.scalar_tensor_tensor(
                out=junk2[:], in0=e[:], scalar=1.0, in1=e[:],
                op0=OP.mult, op1=OP.mult, accum_out=acc9[:, 8:9])

            sseP = pspool.tile([128, 2048], F32, tag="p1ps")
            nc.tensor.matmul(sseP[0:1, 0:1], lhsT=acc9[:, 8:9], rhs=onec[:],
                             start=True, stop=True)
            outsb = cpool.tile([1, 1], F32, tag="outsb")
            nc.vector.tensor_copy(outsb[:], sseP[0:1, 0:1])
            nc.sync.dma_start(out=out_ext[:], in_=outsb[:])

    return nc


def _split_matmul_waits(nc):
    """walrus in this env allows only one sync-wait per engine instruction.
    Hoist extra waits onto same-engine EventSemaphore carriers placed just
    before the instruction in the (already scheduled) stream.  Also expand
    EVENT_SEMAPHORE_RANGE_CLEAR (unsupported encoding) into per-sem writes."""
    cnt = 0
    for fn in nc.m.functions:
        for blk in fn.blocks:
            new = []
            for inst in blk.instructions:
                si = getattr(inst, "sync_info", None)
                eng = getattr(inst, "engine", None)
                if (type(inst).__name__ == "InstISA"
                        and getattr(inst, "op_name", "") ==
                        "EVENT_SEMAPHORE_RANGE_CLEAR"):
                    d = inst.ant_dict
                    waits = list(si.on_wait) if si else []
                    for sid in range(d["range_first"], d["range_last"] + 1):
                        cnt += 1
                        ev = mybir.InstEventSemaphore(name=f"SC-{cnt}")
                        ev.engine = eng
                        ev.sync_info = mybir.SyncInfo(
                            on_wait=[waits.pop()] if waits else [],
                            on_update=[mybir.SyncUpdate(
                                sync_type="semaphore", id=sid,
                                ant_name=f"clear_{sid}",
                                update_mode="sem-wr-imm", update_value=0,
                                update_reg=None)])
                        new.append(ev)
                    while waits:
                        cnt += 1
                        ev = mybir.InstEventSemaphore(name=f"SC-{cnt}")
                        ev.engine = eng
                        ev.sync_info = mybir.SyncInfo(
                            on_wait=[waits.pop()], on_update=[])
                        new.append(ev)
                    continue
                splittable = type(inst).__name__ in (
                    "InstMatmult", "InstActivation", "InstTensorTensor",
                    "InstTensorScalarPtr", "InstTensorTensorReduce",
                    "InstTensorCopy", "InstCustomDveAnt", "InstReciprocal",
                    "InstMemset", "InstTensorReduce", "InstCopy",
                    "InstStreamTranspose", "InstCopyPredicated",
                    "InstDMACopy", "InstDrain")
                if (si is not None and len(si.on_wait) > 1
                        and eng is not None
                        and eng != mybir.EngineType.Unassigned
                        and splittable):
                    waits = list(si.on_wait)
                    for w in waits[:-1]:
                        cnt += 1
                        nop = mybir.InstEventSemaphore(name=f"WN-{cnt}")
                        nop.engine = eng
                        nop.sync_info = mybir.SyncInfo(on_wait=[w], on_update=[])
                        new.append(nop)
                    inst.sync_info = mybir.SyncInfo(
                        on_wait=[waits[-1]], on_update=list(si.on_update))
                new.append(inst)
            blk.instructions = new
    return nc


_NC_CACHE = None


def get_nc():
    global _NC_CACHE
    if _NC_CACHE is None:
        _NC_CACHE = _split_matmul_waits(build_nc())
    return _NC_CACHE


def make_in_maps(I, u, b):
    I = np.ascontiguousarray(np.asarray(I), dtype=np.float32)
    u = np.ascontiguousarray(np.asarray(u), dtype=np.float32)
    b = np.ascontiguousarray(np.asarray(b), dtype=np.float32)
    return [{"I": np.ascontiguousarray(I[i, 0]),
             "u": np.ascontiguousarray(u[i]),
             "b": np.ascontiguousarray(b[i, 0])} for i in range(NCORES)]


def kernel(I, u, b, p, sigma):
    assert int(np.asarray(p)) == 2 and int(np.asarray(sigma)) == 4
    nc = get_nc()
    in_maps = make_in_maps(I, u, b)
    res = run_bass_kernel_spmd(nc, in_maps, list(range(NCORES)))
    sse = sum(float(res.results[i]["out"][0, 0]) for i in range(NCORES))
    loss = np.float64(sse) / (NCORES * H * W)
    return np.array([loss], dtype=np.float32)


if __name__ == "__main__":
    rng = np.random.default_rng(0)
    I = rng.random((8, 1, H, W), dtype=np.float32)
    u = rng.random((8, NCH, H, W), dtype=np.float32)
    b = rng.random((8, 1, H, W), dtype=np.float32) + 0.5
    print(kernel(I, u, b, 2, 4))


# revision 36
# speedup vs baseline: 1.1243x; 1.1243x over previous
"""BiasPredictLoss Trainium2 kernel.

Data-parallel over batch: 8 samples -> 8 NeuronCores, one sample each.
Per core computes the per-sample sum of squared errors (SSE) of
(b - b_new); host averages the 8 scalars.

Math (per sample, K = 17x17 separable Gaussian, sigma=4, p=2):
  mask  = (I > 0)
  r     = 1/(conv(mask)+EPS)            (ACT: exp(-ln(Kb+EPS)), psum-direct)
  t1    = r*I   (mask*I == I);  t2 = r*mask
  A1    = conv(b)*t1 ; A2 = conv(b^2)*t2
  num_c = sum(u_c^2*A1); den_c = sum(u_c^2*A2); v_c = num/(den+EPS)
  w1    = sum_c u_c^2 v_c ; w2 = sum_c u_c^2 v_c^2
  q     = conv(I*w1)/conv(w2)           (valid where mask==1; conv(w2)>0
                                         everywhere since u>0 regardless of mask)
  e     = b - q   where mask==1, else b - 1    (reference's EPS terms are
                                                f32-invisible; masked branch exact)
  SSE   = sum(e^2)

Convolution on TensorE in bf16 (fp32 matmuls decompose 2.3x on trn2):
  conv2(X^T) = Ag @ ((X^T)^T @ Ag) with the image as the stationary operand in
  pass 1 -- conv of a transposed input returns a normal-orientation output, so
  phase-A and phase-B convs all take transposed bf16 inputs and everything else
  stays in normal orientation.
"""

import sys

import numpy as np

for _p in ("/opt/trn_rl_repo",):
    if _p not in sys.path:
        sys.path.insert(0, _p)

import concourse.bass as bass
import concourse.mybir as mybir
from concourse.tile import TileContext
from concourse.bass_utils import run_bass_kernel_spmd

F32 = mybir.dt.float32
BF16 = mybir.dt.bfloat16
OP = mybir.AluOpType
AF = mybir.ActivationFunctionType

EPS = 1e-9
H = W = 512
NCH = 4
NB = 4  # 128-row blocks per image
NCORES = 8
SIG = 4
KS = 4 * SIG + 1
HB = KS // 2


def _toeplitz(dtype):
    ax = np.arange(KS, dtype=np.float64) - (KS - 1) / 2.0
    g = np.exp(-(ax ** 2) / (2.0 * SIG ** 2))
    gn = g / g.sum()
    A = np.zeros((H, H), dtype=np.float64)
    for t in range(-HB, HB + 1):
        v = gn[t + HB]
        idx = np.arange(max(0, -t), min(H, H - t))
        A[idx, idx + t] = v
    return A.astype(dtype)


def _blk(t, j):
    return t[:, j * 512:(j + 1) * 512]


def _sub(t, j, m):
    return t[:, j * 512 + m * 128: j * 512 + m * 128 + 128]


DEBUG_TAPS = False


def build_nc():
    import ml_dtypes
    nc = bass.Bass()
    I_ext = nc.declare_dram_parameter("I", [H, W], F32, isOutput=False)
    u_ext = nc.declare_dram_parameter("u", [NCH, H, W], F32, isOutput=False)
    b_ext = nc.declare_dram_parameter("b", [H, W], F32, isOutput=False)
    out_ext = nc.declare_dram_parameter("out", [1, 1], F32, isOutput=True)

    dbg_exts = {}
    if DEBUG_TAPS:
        for nm, shp, dt in [("d_acc", [128, 9], F32), ("d_nd", [1, 16], F32),
                            ("d_vb", [128, 8], F32), ("d_e", [128, 2048], F32),
                            ("d_r", [128, 2048], F32),
                            ("d_q", [128, 2048], F32)]:
            dbg_exts[nm] = nc.declare_dram_parameter(nm, shp, dt, isOutput=True)

    # register extra const APs used by ACT bias lowering (same pattern as
    # Bass.__init__'s builtins)
    _cm1 = nc.alloc_sbuf_tensor("const-float32-m1", [128, 1], F32)
    nc.gpsimd.memset(_cm1.ap(), -1.0)
    nc.const_aps.aps[(F32, -1.0)] = _cm1.ap()
    nc.all_engine_barrier()

    Ag_d = nc.inline_tensor(_toeplitz(ml_dtypes.bfloat16), name="Ag_const")
    id_d = nc.inline_tensor(np.eye(128, dtype=ml_dtypes.bfloat16),
                            name="id_const")
    onec_d = nc.inline_tensor(np.ones((128, 1), np.float32), name="onec_const")
    onecb_d = nc.inline_tensor(np.ones((128, 1), ml_dtypes.bfloat16),
                               name="onecb_const")
    oner_d = nc.inline_tensor(np.ones((1, 128), np.float32), name="oner_const")

    with TileContext(nc) as tc:
        with tc.tile_pool(name="const", bufs=1) as cpool, \
             tc.tile_pool(name="imgs", bufs=1) as ipool, \
             tc.tile_pool(name="ps", bufs=1, space="PSUM") as pspool:

            # ---- constants to SBUF ----
            Ag = cpool.tile([128, 2048], BF16, tag="Ag")
            nc.sync.dma_start(
                out=Ag[:].rearrange("p (j w) -> p j w", w=512),
                in_=Ag_d[:].rearrange("(j p) w -> p j w", p=128))
            ident = cpool.tile([128, 128], BF16, tag="ident")
            nc.sync.dma_start(out=ident[:], in_=id_d[:])
            onec = cpool.tile([128, 1], F32, tag="onec")
            nc.sync.dma_start(out=onec[:], in_=onec_d[:])
            onecb_raw = cpool.tile([128, 1], BF16, tag="onecb_raw")
            nc.sync.dma_start(out=onecb_raw[:], in_=onecb_d[:])
            onecb = cpool.tile([128, 1], BF16, tag="onecb")
            nc.vector.tensor_copy(onecb[:], onecb_raw[:])
            oner = cpool.tile([1, 128], F32, tag="oner")
            nc.sync.dma_start(out=oner[:], in_=oner_d[:])
            eps_col = cpool.tile([128, 1], F32, tag="eps_col")
            nc.vector.memset(eps_col[:], EPS)

            def tap(nm, tile_ap):
                if DEBUG_TAPS:
                    nc.sync.dma_start(out=dbg_exts[nm][:], in_=tile_ap)

            def _half_conv(X_bf, outtag):
                """one data-stationary pass: out = X^T @ Ag (windowed band)."""
                out = pspool.tile([128, 2048], F32, tag=outtag, name=outtag)
                for m in range(NB):
                    for k in range(NB):
                        n0 = max(0, k * 128 - HB)
                        n1 = min(512, k * 128 + 128 + HB)
                        nc.tensor.matmul(
                            out[:, m * 512 + n0: m * 512 + n1],
                            lhsT=_sub(X_bf, k, m),
                            rhs=Ag[:, k * 512 + n0: k * 512 + n1],
                            start=(k == 0), stop=(k == NB - 1))
                return out

            conv_no = [0]

            def conv2(X_bf, outtag):
                """X normal bf16 -> conv2(X) psum f32, normal orientation.
                P1 = X^T@Ag = (AgX)^T, then out = P1^T@Ag = (AgX)Ag."""
                p1 = _half_conv(X_bf, "p1ps")
                p1sb = ipool.tile([128, 2048], BF16, tag="p1sb")
                if conv_no[0] % 2 == 0:
                    nc.scalar.copy(p1sb[:], p1[:])
                else:
                    nc.vector.tensor_copy(p1sb[:], p1[:])
                conv_no[0] += 1
                return _half_conv(p1sb, outtag)

            # ---- input DMA ----
            I_sb = ipool.tile([128, 2048], F32, tag="I")
            b_sb = ipool.tile([128, 2048], F32, tag="b")
            u_sb = [ipool.tile([128, 2048], F32, tag=f"u{c}", name=f"u{c}")
                    for c in range(NCH)]
            # I first (gates mask -> conv(mask) -> r, the critical chain),
            # then b, then u (only needed from the squares onward)
            for j in range(NB):
                nc.sync.dma_start(out=_blk(I_sb, j), in_=I_ext[j * 128:(j + 1) * 128, :])
            for j in range(NB):
                nc.sync.dma_start(out=_blk(b_sb, j), in_=b_ext[j * 128:(j + 1) * 128, :])
            for c in range(NCH):
                for j in range(NB):
                    nc.sync.dma_start(out=_blk(u_sb[c], j),
                                      in_=u_ext[c, j * 128:(j + 1) * 128, :])

            # ---- bf16 prep + transposes ----
            mask_bf = ipool.tile([128, 2048], BF16, tag="mask_bf")
            nc.scalar.activation(mask_bf[:], I_sb[:], AF.Sign)
            b_bf = ipool.tile([128, 2048], BF16, tag="b_bf")
            nc.vector.tensor_copy(b_bf[:], b_sb[:])

            b2_bf = ipool.tile([128, 2048], BF16, tag="b2_bf")
            nc.scalar.activation(b2_bf[:], b_sb[:], AF.Square)

            # ---- phase A convolutions + r ----
            KbP = conv2(mask_bf, "convout")
            rln = ipool.tile([128, 2048], F32, tag="rln")
            nc.scalar.activation(rln[:], KbP[:], AF.Ln, bias=eps_col[:])
            r_bf = ipool.tile([128, 2048], BF16, tag="r_bf")
            nc.scalar.activation(r_bf[:], rln[:], AF.Exp, scale=-1.0)
            tap('d_r', rln[:])
            t1 = ipool.tile([128, 2048], BF16, tag="t1")
            nc.vector.tensor_mul(t1[:], r_bf[:], I_sb[:])
            t2 = ipool.tile([128, 2048], BF16, tag="t2")
            nc.vector.tensor_mul(t2[:], r_bf[:], mask_bf[:])

            CbP = conv2(b_bf, "convout")
            A1 = ipool.tile([128, 2048], BF16, tag="A1")
            nc.vector.tensor_mul(A1[:], CbP[:], t1[:])
            Cb2P = conv2(b2_bf, "convout")
            A2 = ipool.tile([128, 2048], BF16, tag="A2")
            nc.vector.tensor_mul(A2[:], Cb2P[:], t2[:])

            # ---- u squares (bf16) ----
            s_sb = []
            for c in range(NCH):
                s = ipool.tile([128, 2048], BF16, tag=f"s{c}", name=f"s{c}")
                nc.scalar.activation(s[:], u_sb[c][:], AF.Square)
                s_sb.append(s)

            # ---- class-center reductions ----
            # nf_c = s_c * A (bf16 TT, 2x) then TensorE ones-matmul reduces
            # partitions into [1,512] psum rows (keeps PE warm mid-kernel);
            # one batched 3D tensor_reduce per psum tile finishes the job.
            acc9 = cpool.tile([128, 9], F32, tag="acc9")
            nd = cpool.tile([1, 16], F32, tag="nd")
            junk = ipool.tile([128, 2048], BF16, tag="junk")
            for c in range(NCH):
                nc.vector.scalar_tensor_tensor(
                    out=junk[:], in0=s_sb[c][:], scalar=1.0, in1=A1[:],
                    op0=OP.mult, op1=OP.mult, accum_out=acc9[:, c:c + 1])
            for c in range(NCH):
                nc.vector.scalar_tensor_tensor(
                    out=junk[:], in0=s_sb[c][:], scalar=1.0, in1=A2[:],
                    op0=OP.mult, op1=OP.mult, accum_out=acc9[:, 4 + c:5 + c])
            ndP = pspool.tile([128, 2048], F32, tag="p1ps")
            nc.tensor.matmul(ndP[0:1, 0:8], lhsT=onec[:], rhs=acc9[:, 0:8],
                             start=True, stop=True)
            nc.vector.tensor_copy(nd[0:1, 0:8], ndP[0:1, 0:8])
            nc.vector.tensor_scalar_add(nd[0:1, 4:8], nd[0:1, 4:8], EPS)
            nc.vector.reciprocal(nd[0:1, 8:12], nd[0:1, 4:8])
            nc.vector.tensor_mul(nd[0:1, 12:16], nd[0:1, 0:4], nd[0:1, 8:12])
            tap('d_nd', nd[:])
            tap('d_acc', acc9[:])
            vcat = cpool.tile([1, 8], F32, tag="vcat")
            nc.vector.tensor_copy(vcat[0:1, 0:4], nd[0:1, 12:16])
            nc.vector.tensor_mul(vcat[0:1, 4:8], nd[0:1, 12:16], nd[0:1, 12:16])

            vbP = pspool.tile([128, 2048], F32, tag="convout")
            nc.tensor.matmul(vbP[:, 0:8], lhsT=oner[:], rhs=vcat[:],
                             start=True, stop=True)
            vb = cpool.tile([128, 8], F32, tag="vb")
            nc.vector.tensor_copy(vb[:], vbP[:, 0:8])
            tap('d_vb', vb[:])
            vId = cpool.tile([128, 1024], BF16, tag="vId")
            for c in range(8):
                nc.vector.tensor_scalar_mul(vId[:, c * 128:(c + 1) * 128],
                                            ident[:], vb[:, c:c + 1])

            # ---- w1 / w2 -> phase-B conv inputs (transposed bf16) ----
            w1P = pspool.tile([128, 2048], F32, tag="p1ps")
            for j in range(NB):
                for c in range(NCH):
                    nc.tensor.matmul(_blk(w1P, j), lhsT=vId[:, c * 128:(c + 1) * 128],
                                     rhs=_blk(s_sb[c], j),
                                     start=(c == 0), stop=(c == 3))
            X1 = ipool.tile([128, 2048], BF16, tag="X1")
            nc.vector.tensor_mul(X1[:], w1P[:], I_sb[:])
            w2P = pspool.tile([128, 2048], F32, tag="convout")
            for j in range(NB):
                for c in range(NCH):
                    nc.tensor.matmul(_blk(w2P, j),
                                     lhsT=vId[:, 512 + c * 128: 512 + (c + 1) * 128],
                                     rhs=_blk(s_sb[c], j),
                                     start=(c == 0), stop=(c == 3))
            X2 = ipool.tile([128, 2048], BF16, tag="X2")
            nc.any.tensor_copy(X2[:], w2P[:])

            # ---- phase B ----
            C2P = conv2(X2, "convout")
            dln = ipool.tile([128, 2048], F32, tag="dln")
            nc.scalar.activation(dln[:], C2P[:], AF.Ln)
            rDB = ipool.tile([128, 2048], F32, tag="rDB")
            nc.scalar.activation(rDB[:], dln[:], AF.Exp, scale=-1.0)
            C1P = conv2(X1, "convout")
            q = ipool.tile([128, 2048], F32, tag="q")
            nc.vector.tensor_mul(q[:], C1P[:], rDB[:])
            tap('d_q', q[:])

            e = ipool.tile([128, 2048], F32, tag="e")
            nc.vector.tensor_sub(e[:], b_sb[:], q[:])
            # masked-out pixels: e = b - 1 exactly
            z_bf = ipool.tile([128, 2048], mybir.dt.uint8, tag="z_bf")
            nc.vector.tensor_scalar(z_bf[:], mask_bf[:], 0.0, None,
                                    OP.is_equal)
            bm1 = ipool.tile([128, 2048], F32, tag="bm1")
            nc.scalar.add(bm1[:], b_sb[:], -1.0)
            nc.vector.copy_predicated(e[:], z_bf[:], bm1[:])
            tap('d_e', e[:])

            junk2 = ipool.tile([128, 2048], F32, tag="junk2")
            nc.vector.scalar_tensor_tensor(
                out=junk2[:], in0=e[:], scalar=1.0, in1=e[:],
                op0=OP.mult, op1=OP.mult, accum_out=acc9[:, 8:9])

            sseP = pspool.tile([128, 2048], F32, tag="p1ps")
            nc.tensor.matmul(sseP[0:1, 0:1], lhsT=acc9[:, 8:9], rhs=onec[:],
                             start=True, stop=True)
            outsb = cpool.tile([1, 1], F32, tag="outsb")
            nc.vector.tensor_copy(outsb[:], sseP[0:1, 0:1])
            nc.sync.dma_start(out=out_ext[:], in_=outsb[:])

    return nc


def _split_matmul_waits(nc):
    """walrus in this env allows only one sync-wait per engine instruction.
    Hoist extra waits onto same-engine EventSemaphore carriers placed just
    before the instruction in the (already scheduled) stream.  Also expand
    EVENT_SEMAPHORE_RANGE_CLEAR (unsupported encoding) into per-sem writes."""
    cnt = 0
    for fn in nc.m.functions:
        for blk in fn.blocks:
            new = []
            for inst in blk.instructions:
                si = getattr(inst, "sync_info", None)
                eng = getattr(inst, "engine", None)
                if (type(inst).__name__ == "InstISA"
                        and getattr(inst, "op_name", "") ==
                        "EVENT_SEMAPHORE_RANGE_CLEAR"):
                    d = inst.ant_dict
                    waits = list(si.on_wait) if si else []
                    for sid in range(d["range_first"], d["range_last"] + 1):
                        cnt += 1
                        ev = mybir.InstEventSemaphore(name=f"SC-{cnt}")
                        ev.engine = eng
                        ev.sync_info = mybir.SyncInfo(
                            on_wait=[waits.pop()] if waits else [],
                            on_update=[mybir.SyncUpdate(
                                sync_type="semaphore", id=sid,
                                ant_name=f"clear_{sid}",
                                update_mode="sem-wr-imm", update_value=0,
                                update_reg=None)])
                        new.append(ev)
                    while waits:
                        cnt += 1
                        ev = mybir.InstEventSemaphore(name=f"SC-{cnt}")
                        ev.engine = eng
                        ev.sync_info = mybir.SyncInfo(
                            on_wait=[waits.pop()], on_update=[])
                        new.append(ev)
                    continue
                splittable = type(inst).__name__ in (
                    "InstMatmult", "InstActivation", "InstTensorTensor",
                    "InstTensorScalarPtr", "InstTensorTensorReduce",
                    "InstTensorCopy", "InstCustomDveAnt", "InstReciprocal",
                    "InstMemset", "InstTensorReduce", "InstCopy",
                    "InstStreamTranspose", "InstCopyPredicated",
                    "InstDMACopy", "InstDrain")
                if (si is not None and len(si.on_wait) > 1
                        and eng is not None
                        and eng != mybir.EngineType.Unassigned
                        and splittable):
                    waits = list(si.on_wait)
                    for w in waits[:-1]:
                        cnt += 1
                        nop = mybir.InstEventSemaphore(name=f"WN-{cnt}")
                        nop.engine = eng
                        nop.sync_info = mybir.SyncInfo(on_wait=[w], on_update=[])
                        new.append(nop)
                    inst.sync_info = mybir.SyncInfo(
                        on_wait=[waits[-1]], on_update=list(si.on_update))
                new.append(inst)
            blk.instructions = new
    return nc


_NC_CACHE = None


def get_nc():
    global _NC_CACHE
    if _NC_CACHE is None:
        _NC_CACHE = _split_matmul_waits(build_nc())
    return _NC_CACHE


def make_in_maps(I, u, b):
    I = np.ascontiguousarray(np.asarray(I), dtype=np.float32)
    u = np.ascontiguousarray(np.asarray(u), dtype=np.float32)
    b = np.ascontiguousarray(np.asarray(b), dtype=np.float32)
    return [{"I": np.ascontiguousarray(I[i, 0]),
             "u": np.ascontiguousarray(u[i]),
             "b": np.ascontiguousarray(b[i, 0])} for i in range(NCORES)]


def kernel(I, u, b, p, sigma):
    assert int(np.asarray(p)) == 2 and int(np.asarray(sigma)) == 4
    nc = get_nc()
    in_maps = make_in_maps(I, u, b)
    res = run_bass_kernel_spmd(nc, in_maps, list(range(NCORES)))
    sse = sum(float(res.results[i]["out"][0, 0]) for i in range(NCORES))
    loss = np.float64(sse) / (NCORES * H * W)
    return np.array([loss], dtype=np.float32)


if __name__ == "__main__":
    rng = np.random.default_rng(0)
    I = rng.random((8, 1, H, W), dtype=np.float32)
    u = rng.random((8, NCH, H, W), dtype=np.float32)
    b = rng.random((8, 1, H, W), dtype=np.float32) + 0.5
    print(kernel(I, u, b, 2, 4))


# revision 37
# speedup vs baseline: 1.1838x; 1.0529x over previous
"""BiasPredictLoss Trainium2 kernel.

Data-parallel over batch: 8 samples -> 8 NeuronCores, one sample each.
Per core computes the per-sample sum of squared errors (SSE) of
(b - b_new); host averages the 8 scalars.

Math (per sample, K = 17x17 separable Gaussian, sigma=4, p=2):
  mask  = (I > 0)
  r     = 1/(conv(mask)+EPS)            (ACT: exp(-ln(Kb+EPS)), psum-direct)
  t1    = r*I   (mask*I == I);  t2 = r*mask
  A1    = conv(b)*t1 ; A2 = conv(b^2)*t2
  num_c = sum(u_c^2*A1); den_c = sum(u_c^2*A2); v_c = num/(den+EPS)
  w1    = sum_c u_c^2 v_c ; w2 = sum_c u_c^2 v_c^2
  q     = conv(I*w1)/conv(w2)           (valid where mask==1; conv(w2)>0
                                         everywhere since u>0 regardless of mask)
  e     = b - q   where mask==1, else b - 1    (reference's EPS terms are
                                                f32-invisible; masked branch exact)
  SSE   = sum(e^2)

Convolution on TensorE in bf16 (fp32 matmuls decompose 2.3x on trn2):
  conv2(X^T) = Ag @ ((X^T)^T @ Ag) with the image as the stationary operand in
  pass 1 -- conv of a transposed input returns a normal-orientation output, so
  phase-A and phase-B convs all take transposed bf16 inputs and everything else
  stays in normal orientation.
"""

import sys

import numpy as np

for _p in ("/opt/trn_rl_repo",):
    if _p not in sys.path:
        sys.path.insert(0, _p)

import concourse.bass as bass
import concourse.mybir as mybir
from concourse.tile import TileContext
from concourse.bass_utils import run_bass_kernel_spmd

F32 = mybir.dt.float32
BF16 = mybir.dt.bfloat16
OP = mybir.AluOpType
AF = mybir.ActivationFunctionType

EPS = 1e-9
H = W = 512
NCH = 4
NB = 4  # 128-row blocks per image
NCORES = 8
SIG = 4
KS = 4 * SIG + 1
HB = KS // 2


def _toeplitz(dtype):
    ax = np.arange(KS, dtype=np.float64) - (KS - 1) / 2.0
    g = np.exp(-(ax ** 2) / (2.0 * SIG ** 2))
    gn = g / g.sum()
    A = np.zeros((H, H), dtype=np.float64)
    for t in range(-HB, HB + 1):
        v = gn[t + HB]
        idx = np.arange(max(0, -t), min(H, H - t))
        A[idx, idx + t] = v
    return A.astype(dtype)


def _blk(t, j):
    return t[:, j * 512:(j + 1) * 512]


def _sub(t, j, m):
    return t[:, j * 512 + m * 128: j * 512 + m * 128 + 128]


DEBUG_TAPS = False


def build_nc():
    import ml_dtypes
    nc = bass.Bass()
    I_ext = nc.declare_dram_parameter("I", [H, W], F32, isOutput=False)
    u_ext = nc.declare_dram_parameter("u", [NCH, H, W], F32, isOutput=False)
    b_ext = nc.declare_dram_parameter("b", [H, W], F32, isOutput=False)
    out_ext = nc.declare_dram_parameter("out", [1, 1], F32, isOutput=True)

    dbg_exts = {}
    if DEBUG_TAPS:
        for nm, shp, dt in [("d_acc", [128, 9], F32), ("d_nd", [1, 16], F32),
                            ("d_vb", [128, 8], F32), ("d_e", [128, 2048], F32),
                            ("d_r", [128, 2048], F32),
                            ("d_q", [128, 2048], F32)]:
            dbg_exts[nm] = nc.declare_dram_parameter(nm, shp, dt, isOutput=True)

    # register extra const APs used by ACT bias lowering (same pattern as
    # Bass.__init__'s builtins)
    _cm1 = nc.alloc_sbuf_tensor("const-float32-m1", [128, 1], F32)
    nc.gpsimd.memset(_cm1.ap(), -1.0)
    nc.const_aps.aps[(F32, -1.0)] = _cm1.ap()
    nc.all_engine_barrier()

    Ag_d = nc.inline_tensor(_toeplitz(ml_dtypes.bfloat16), name="Ag_const")
    id_d = nc.inline_tensor(np.eye(128, dtype=ml_dtypes.bfloat16),
                            name="id_const")
    onec_d = nc.inline_tensor(np.ones((128, 1), np.float32), name="onec_const")
    onecb_d = nc.inline_tensor(np.ones((128, 1), ml_dtypes.bfloat16),
                               name="onecb_const")
    oner_d = nc.inline_tensor(np.ones((1, 128), np.float32), name="oner_const")

    with TileContext(nc) as tc:
        with tc.tile_pool(name="const", bufs=1) as cpool, \
             tc.tile_pool(name="imgs", bufs=1) as ipool, \
             tc.tile_pool(name="ps", bufs=1, space="PSUM") as pspool:

            # ---- constants to SBUF ----
            Ag = cpool.tile([128, 2048], BF16, tag="Ag")
            nc.sync.dma_start(
                out=Ag[:].rearrange("p (j w) -> p j w", w=512),
                in_=Ag_d[:].rearrange("(j p) w -> p j w", p=128))
            ident = cpool.tile([128, 128], BF16, tag="ident")
            nc.sync.dma_start(out=ident[:], in_=id_d[:])
            onec = cpool.tile([128, 1], F32, tag="onec")
            nc.sync.dma_start(out=onec[:], in_=onec_d[:])
            onecb_raw = cpool.tile([128, 1], BF16, tag="onecb_raw")
            nc.sync.dma_start(out=onecb_raw[:], in_=onecb_d[:])
            onecb = cpool.tile([128, 1], BF16, tag="onecb")
            nc.vector.tensor_copy(onecb[:], onecb_raw[:])
            oner = cpool.tile([1, 128], F32, tag="oner")
            nc.sync.dma_start(out=oner[:], in_=oner_d[:])
            eps_col = cpool.tile([128, 1], F32, tag="eps_col")
            nc.vector.memset(eps_col[:], EPS)

            def tap(nm, tile_ap):
                if DEBUG_TAPS:
                    nc.sync.dma_start(out=dbg_exts[nm][:], in_=tile_ap)

            def _half_conv(X_bf, outtag):
                """one data-stationary pass: out = X^T @ Ag (windowed band)."""
                out = pspool.tile([128, 2048], F32, tag=outtag, name=outtag)
                for m in range(NB):
                    for k in range(NB):
                        n0 = max(0, k * 128 - HB)
                        n1 = min(512, k * 128 + 128 + HB)
                        nc.tensor.matmul(
                            out[:, m * 512 + n0: m * 512 + n1],
                            lhsT=_sub(X_bf, k, m),
                            rhs=Ag[:, k * 512 + n0: k * 512 + n1],
                            start=(k == 0), stop=(k == NB - 1))
                return out

            conv_no = [0]

            def conv2(X_bf, outtag):
                """X normal bf16 -> conv2(X) psum f32, normal orientation.
                P1 = X^T@Ag = (AgX)^T, then out = P1^T@Ag = (AgX)Ag."""
                p1 = _half_conv(X_bf, "p1ps")
                p1sb = ipool.tile([128, 2048], BF16, tag="p1sb")
                if conv_no[0] % 2 == 0:
                    nc.scalar.copy(p1sb[:], p1[:])
                else:
                    nc.vector.tensor_copy(p1sb[:], p1[:])
                conv_no[0] += 1
                return _half_conv(p1sb, outtag)

            # ---- input DMA ----
            I_sb = ipool.tile([128, 2048], F32, tag="I")
            b_sb = ipool.tile([128, 2048], F32, tag="b")
            u_sb = [ipool.tile([128, 2048], F32, tag=f"u{c}", name=f"u{c}")
                    for c in range(NCH)]
            # I first (gates mask -> conv(mask) -> r, the critical chain),
            # then b, then u (only needed from the squares onward)
            for j in range(NB):
                nc.sync.dma_start(out=_blk(I_sb, j), in_=I_ext[j * 128:(j + 1) * 128, :])
            for j in range(NB):
                nc.sync.dma_start(out=_blk(b_sb, j), in_=b_ext[j * 128:(j + 1) * 128, :])
            for c in range(NCH):
                for j in range(NB):
                    nc.sync.dma_start(out=_blk(u_sb[c], j),
                                      in_=u_ext[c, j * 128:(j + 1) * 128, :])

            # ---- bf16 prep (chunked per h-block so conv pass1 can start
            #      as soon as the first DMA block lands) ----
            mask_bf = ipool.tile([128, 2048], BF16, tag="mask_bf")
            b_bf = ipool.tile([128, 2048], BF16, tag="b_bf")
            b2_bf = ipool.tile([128, 2048], BF16, tag="b2_bf")
            for j in range(NB):
                nc.scalar.activation(_blk(mask_bf, j), _blk(I_sb, j), AF.Sign)
            for j in range(NB):
                nc.vector.tensor_copy(_blk(b_bf, j), _blk(b_sb, j))
            for j in range(NB):
                nc.scalar.activation(_blk(b2_bf, j), _blk(b_sb, j), AF.Square)

            # ---- phase A convolutions + r ----
            KbP = conv2(mask_bf, "convout")
            rln = ipool.tile([128, 2048], F32, tag="rln")
            nc.scalar.activation(rln[:], KbP[:], AF.Ln, bias=eps_col[:])
            r_bf = ipool.tile([128, 2048], BF16, tag="r_bf")
            nc.scalar.activation(r_bf[:], rln[:], AF.Exp, scale=-1.0)
            tap('d_r', rln[:])
            t1 = ipool.tile([128, 2048], BF16, tag="t1")
            nc.vector.tensor_mul(t1[:], r_bf[:], I_sb[:])
            t2 = ipool.tile([128, 2048], BF16, tag="t2")
            nc.vector.tensor_mul(t2[:], r_bf[:], mask_bf[:])

            CbP = conv2(b_bf, "convout")
            A1 = ipool.tile([128, 2048], BF16, tag="A1")
            nc.vector.tensor_mul(A1[:], CbP[:], t1[:])
            Cb2P = conv2(b2_bf, "convout")
            A2 = ipool.tile([128, 2048], BF16, tag="A2")
            nc.vector.tensor_mul(A2[:], Cb2P[:], t2[:])

            # ---- u squares (bf16) ----
            s_sb = []
            for c in range(NCH):
                s = ipool.tile([128, 2048], BF16, tag=f"s{c}", name=f"s{c}")
                nc.scalar.activation(s[:], u_sb[c][:], AF.Square)
                s_sb.append(s)

            # ---- class-center reductions ----
            # nf_c = s_c * A (bf16 TT, 2x) then TensorE ones-matmul reduces
            # partitions into [1,512] psum rows (keeps PE warm mid-kernel);
            # one batched 3D tensor_reduce per psum tile finishes the job.
            acc9 = cpool.tile([128, 9], F32, tag="acc9")
            nd = cpool.tile([1, 16], F32, tag="nd")
            junk = ipool.tile([128, 2048], BF16, tag="junk")
            for c in range(NCH):
                nc.vector.scalar_tensor_tensor(
                    out=junk[:], in0=s_sb[c][:], scalar=1.0, in1=A1[:],
                    op0=OP.mult, op1=OP.mult, accum_out=acc9[:, c:c + 1])
            for c in range(NCH):
                nc.vector.scalar_tensor_tensor(
                    out=junk[:], in0=s_sb[c][:], scalar=1.0, in1=A2[:],
                    op0=OP.mult, op1=OP.mult, accum_out=acc9[:, 4 + c:5 + c])
            ndP = pspool.tile([128, 2048], F32, tag="p1ps")
            nc.tensor.matmul(ndP[0:1, 0:8], lhsT=onec[:], rhs=acc9[:, 0:8],
                             start=True, stop=True)
            nc.vector.tensor_copy(nd[0:1, 0:8], ndP[0:1, 0:8])
            nc.vector.tensor_scalar_add(nd[0:1, 4:8], nd[0:1, 4:8], EPS)
            nc.vector.reciprocal(nd[0:1, 8:12], nd[0:1, 4:8])
            nc.vector.tensor_mul(nd[0:1, 12:16], nd[0:1, 0:4], nd[0:1, 8:12])
            tap('d_nd', nd[:])
            tap('d_acc', acc9[:])
            vcat = cpool.tile([1, 8], F32, tag="vcat")
            nc.vector.tensor_copy(vcat[0:1, 0:4], nd[0:1, 12:16])
            nc.vector.tensor_mul(vcat[0:1, 4:8], nd[0:1, 12:16], nd[0:1, 12:16])

            vbP = pspool.tile([128, 2048], F32, tag="convout")
            nc.tensor.matmul(vbP[:, 0:8], lhsT=oner[:], rhs=vcat[:],
                             start=True, stop=True)
            vb = cpool.tile([128, 8], F32, tag="vb")
            nc.vector.tensor_copy(vb[:], vbP[:, 0:8])
            tap('d_vb', vb[:])
            vId = cpool.tile([128, 1024], BF16, tag="vId")
            for c in range(8):
                nc.vector.tensor_scalar_mul(vId[:, c * 128:(c + 1) * 128],
                                            ident[:], vb[:, c:c + 1])

            # ---- w1 / w2 -> phase-B conv inputs (transposed bf16) ----
            w1P = pspool.tile([128, 2048], F32, tag="p1ps")
            for j in range(NB):
                for c in range(NCH):
                    nc.tensor.matmul(_blk(w1P, j), lhsT=vId[:, c * 128:(c + 1) * 128],
                                     rhs=_blk(s_sb[c], j),
                                     start=(c == 0), stop=(c == 3))
            X1 = ipool.tile([128, 2048], BF16, tag="X1")
            nc.vector.tensor_mul(X1[:], w1P[:], I_sb[:])
            w2P = pspool.tile([128, 2048], F32, tag="convout")
            for j in range(NB):
                for c in range(NCH):
                    nc.tensor.matmul(_blk(w2P, j),
                                     lhsT=vId[:, 512 + c * 128: 512 + (c + 1) * 128],
                                     rhs=_blk(s_sb[c], j),
                                     start=(c == 0), stop=(c == 3))
            X2 = ipool.tile([128, 2048], BF16, tag="X2")
            nc.any.tensor_copy(X2[:], w2P[:])

            # ---- phase B ----
            C2P = conv2(X2, "convout")
            dln = ipool.tile([128, 2048], F32, tag="dln")
            nc.scalar.activation(dln[:], C2P[:], AF.Ln)
            rDB = ipool.tile([128, 2048], F32, tag="rDB")
            nc.scalar.activation(rDB[:], dln[:], AF.Exp, scale=-1.0)
            C1P = conv2(X1, "convout")
            q = ipool.tile([128, 2048], F32, tag="q")
            nc.vector.tensor_mul(q[:], C1P[:], rDB[:])
            tap('d_q', q[:])

            e = ipool.tile([128, 2048], F32, tag="e")
            nc.vector.tensor_sub(e[:], b_sb[:], q[:])
            # masked-out pixels: e = b - 1 exactly
            z_bf = ipool.tile([128, 2048], mybir.dt.uint8, tag="z_bf")
            nc.vector.tensor_scalar(z_bf[:], mask_bf[:], 0.0, None,
                                    OP.is_equal)
            bm1 = ipool.tile([128, 2048], F32, tag="bm1")
            nc.scalar.add(bm1[:], b_sb[:], -1.0)
            nc.vector.copy_predicated(e[:], z_bf[:], bm1[:])
            tap('d_e', e[:])

            junk2 = ipool.tile([128, 2048], F32, tag="junk2")
            nc.vector.scalar_tensor_tensor(
                out=junk2[:], in0=e[:], scalar=1.0, in1=e[:],
                op0=OP.mult, op1=OP.mult, accum_out=acc9[:, 8:9])

            sseP = pspool.tile([128, 2048], F32, tag="p1ps")
            nc.tensor.matmul(sseP[0:1, 0:1], lhsT=acc9[:, 8:9], rhs=onec[:],
                             start=True, stop=True)
            outsb = cpool.tile([1, 1], F32, tag="outsb")
            nc.vector.tensor_copy(outsb[:], sseP[0:1, 0:1])
            nc.sync.dma_start(out=out_ext[:], in_=outsb[:])

    return nc


def _split_matmul_waits(nc):
    """walrus in this env allows only one sync-wait per engine instruction.
    Hoist extra waits onto same-engine EventSemaphore carriers placed just
    before the instruction in the (already scheduled) stream.  Also expand
    EVENT_SEMAPHORE_RANGE_CLEAR (unsupported encoding) into per-sem writes."""
    cnt = 0
    for fn in nc.m.functions:
        for blk in fn.blocks:
            new = []
            for inst in blk.instructions:
                si = getattr(inst, "sync_info", None)
                eng = getattr(inst, "engine", None)
                if (type(inst).__name__ == "InstISA"
                        and getattr(inst, "op_name", "") ==
                        "EVENT_SEMAPHORE_RANGE_CLEAR"):
                    d = inst.ant_dict
                    waits = list(si.on_wait) if si else []
                    for sid in range(d["range_first"], d["range_last"] + 1):
                        cnt += 1
                        ev = mybir.InstEventSemaphore(name=f"SC-{cnt}")
                        ev.engine = eng
                        ev.sync_info = mybir.SyncInfo(
                            on_wait=[waits.pop()] if waits else [],
                            on_update=[mybir.SyncUpdate(
                                sync_type="semaphore", id=sid,
                                ant_name=f"clear_{sid}",
                                update_mode="sem-wr-imm", update_value=0,
                                update_reg=None)])
                        new.append(ev)
                    while waits:
                        cnt += 1
                        ev = mybir.InstEventSemaphore(name=f"SC-{cnt}")
                        ev.engine = eng
                        ev.sync_info = mybir.SyncInfo(
                            on_wait=[waits.pop()], on_update=[])
                        new.append(ev)
                    continue
                splittable = type(inst).__name__ in (
                    "InstMatmult", "InstActivation", "InstTensorTensor",
                    "InstTensorScalarPtr", "InstTensorTensorReduce",
                    "InstTensorCopy", "InstCustomDveAnt", "InstReciprocal",
                    "InstMemset", "InstTensorReduce", "InstCopy",
                    "InstStreamTranspose", "InstCopyPredicated",
                    "InstDMACopy", "InstDrain")
                if (si is not None and len(si.on_wait) > 1
                        and eng is not None
                        and eng != mybir.EngineType.Unassigned
                        and splittable):
                    waits = list(si.on_wait)
                    for w in waits[:-1]:
                        cnt += 1
                        nop = mybir.InstEventSemaphore(name=f"WN-{cnt}")
                        nop.engine = eng
                        nop.sync_info = mybir.SyncInfo(on_wait=[w], on_update=[])
                        new.append(nop)
                    inst.sync_info = mybir.SyncInfo(
                        on_wait=[waits[-1]], on_update=list(si.on_update))
                new.append(inst)
            blk.instructions = new
    return nc


_NC_CACHE = None


def get_nc():
    global _NC_CACHE
    if _NC_CACHE is None:
        _NC_CACHE = _split_matmul_waits(build_nc())
    return _NC_CACHE


def make_in_maps(I, u, b):
    I = np.ascontiguousarray(np.asarray(I), dtype=np.float32)
    u = np.ascontiguousarray(np.asarray(u), dtype=np.float32)
    b = np.ascontiguousarray(np.asarray(b), dtype=np.float32)
    return [{"I": np.ascontiguousarray(I[i, 0]),
             "u": np.ascontiguousarray(u[i]),
             "b": np.ascontiguousarray(b[i, 0])} for i in range(NCORES)]


def kernel(I, u, b, p, sigma):
    assert int(np.asarray(p)) == 2 and int(np.asarray(sigma)) == 4
    nc = get_nc()
    in_maps = make_in_maps(I, u, b)
    res = run_bass_kernel_spmd(nc, in_maps, list(range(NCORES)))
    sse = sum(float(res.results[i]["out"][0, 0]) for i in range(NCORES))
    loss = np.float64(sse) / (NCORES * H * W)
    return np.array([loss], dtype=np.float32)


if __name__ == "__main__":
    rng = np.random.default_rng(0)
    I = rng.random((8, 1, H, W), dtype=np.float32)
    u = rng.random((8, NCH, H, W), dtype=np.float32)
    b = rng.random((8, 1, H, W), dtype=np.float32) + 0.5
    print(kernel(I, u, b, 2, 4))
